# revision 1
# baseline (speedup 1.0000x reference)
"""Trainium2 Bass kernel for nn_BatchSoftmaxNomax (batch contrastive softmax loss).

Math: scores[b,c,n,f] = <ner[b,n,:], face[c,f,:]>, logits = scores.mean((n,f)),
loss = -mean_b log_softmax(logits)[b,b].
Since the span-means are linear, logits[b,c] = <mean_n ner[b], mean_f face[c]>,
so the O(B^2*N^2*D) einsum collapses to two mean-reductions + a [B,D]x[D,B] matmul.

Sharding (8 cores, batch-sharded), two launches with a host-side gather between
them (a device AllGather works but costs 35-60us of cross-rank barrier/launch-skew
wait through this runtime, dwarfing the ~5us of exchanged data; two independent
launches have no cross-core waits at all):

Launch A (per core, 32 batch rows): stream the bf16-cast ner/face slices (1 MB
each; the cast costs only ~2e-5 rel on the loss since the mean-of-32 averages the
quantization noise away) through both HWDGE rings, span-mean them with 8
accumulating PE matmuls per tensor against a 1/32 selection matrix, and emit the
fp32 means fm/nm [32, 512].
Host: gathers/transposes the 8 cores' means into fmt_full [128 d', 4 k, 256 c]
(bf16) and per-core nmt [128, 4*32], and takes the 256 diagonal dot products.
Launch B (per core): 4 accumulating bf16 matmuls give its [32, 256] logit rows in
PSUM; ACT exp with fused row-sum accumulate gives sum_c exp(logits) per row.
Host: loss = -mean(diag - log(rowsum)).
"""

import ml_dtypes
import numpy as np
from contextlib import ExitStack

B = 256      # global batch
N1 = 32      # ner spans
N2 = 32      # face spans
D = 512      # embed dim
M = 8        # cores
BL = B // M  # local batch rows per core (32)
R = BL * N1  # rows of the flattened local slice (1024)
PJ = R // 128  # rows per partition in the [128, PJ, D] DMA view (8)
NCH = 4      # DMA chunks per input tensor
JPC = PJ // NCH
KD = D // 128  # d-chunks (4)

_CACHE = {}


def _emit_a(ctx, tc, fm_out, nm_out, ner, face, sel):
    from concourse import mybir

    nc = tc.nc
    f32 = mybir.dt.float32
    bf16 = mybir.dt.bfloat16

    consts = ctx.enter_context(tc.tile_pool(name="consts", bufs=1))
    chunks = ctx.enter_context(tc.tile_pool(name="chunks", bufs=4))
    sbuf = ctx.enter_context(tc.tile_pool(name="work", bufs=1))
    mpsum = ctx.enter_context(tc.tile_pool(name="mpsum", bufs=2, space="PSUM"))

    sel_sb = consts.tile([128, BL], bf16)
    nc.sync.dma_start(sel_sb[:], sel)

    # ner/face arrive pre-cast to bf16 by the host (costs 1.7e-5 rel on the loss:
    # the mean-of-32 averages quantization noise away) — halves the stream bytes.
    # Alternate the two HWDGE rings and keep every chunk in flight at once.
    queues = [nc.sync, nc.scalar]

    def mean_t(src_ap, out_dram, tag, qsel):
        # psum[m, d] = sum_{p,j} sel[p, m] * src[8p + j, d] = (1/32) sum_n src[32m + n, d]
        view = src_ap.rearrange("(p j) d -> p j d", p=128)
        ps = mpsum.tile([BL, D], f32, tag="mean", bufs=2)
        for q in range(NCH):
            t = chunks.tile([128, JPC, D], bf16, tag="chunk", bufs=2 * NCH)
            queues[(qsel + q) % 2].dma_start(t[:], view[:, q * JPC:(q + 1) * JPC, :])
            for jj in range(JPC):
                j = q * JPC + jj
                nc.tensor.matmul(
                    ps[:], sel_sb[:], t[:, jj, :],
                    start=(j == 0), stop=(j == PJ - 1),
                )
        mn = sbuf.tile([BL, D], f32, tag="mean_sb_" + tag)
        nc.vector.tensor_copy(mn[:], ps[:])
        nc.sync.dma_start(out_dram, mn[:])

    mean_t(face, fm_out, "fm", 0)
    mean_t(ner, nm_out, "nm", 1)


def _emit_b(ctx, tc, out, fmt_full, nmt):
    from concourse import mybir

    nc = tc.nc
    f32 = mybir.dt.float32
    bf16 = mybir.dt.bfloat16
    AF = mybir.ActivationFunctionType

    sbuf = ctx.enter_context(tc.tile_pool(name="work", bufs=1))
    lpsum = ctx.enter_context(tc.tile_pool(name="lpsum", bufs=1, space="PSUM"))

    # Warm the ACT exp table set while DMAs stream.
    warm_in = sbuf.tile([1, 1], f32)
    nc.vector.memset(warm_in[:], 0.0)
    warm_out = sbuf.tile([1, 1], f32)
    nc.scalar.activation(warm_out[:], warm_in[:], AF.Exp)

    # bf16 means (host-cast) halve the exchange load; split across both rings.
    nt = sbuf.tile([128, KD * BL], bf16)
    nc.scalar.dma_start(nt[:], nmt)
    ff = sbuf.tile([128, KD, B], bf16)
    half = KD // 2
    nc.sync.dma_start(ff[:, :half, :], fmt_full[:, :half, :])
    nc.scalar.dma_start(ff[:, half:, :], fmt_full[:, half:, :])

    lg = lpsum.tile([BL, B], f32)
    for k in range(KD):
        nc.tensor.matmul(
            lg[:], nt[:, k * BL:(k + 1) * BL], ff[:, k, :],
            start=(k == 0), stop=(k == KD - 1),
        )

    # out[:, 0] = sum_c exp(logits[b, c]) via ACT fused row-accumulate.
    # (The 256 diagonal logits are a dot product of stage-A outputs; host does those.)
    # The output is padded to 128 f32/row: a [32, 1] DMA writes 4 B per partition
    # line, and sub-512B HBM writes pay a ~2x read-modify-write completion penalty.
    res = sbuf.tile([BL, 128], f32)
    nc.vector.memset(res[:], 0.0)
    e_sb = sbuf.tile([BL, B], f32)
    nc.scalar.activation(e_sb[:], lg[:], AF.Exp, accum_out=res[:, 0:1])
    nc.sync.dma_start(out, res[:])


def _build_a():
    import concourse.tile as tile
    from concourse import bacc, mybir

    f32 = mybir.dt.float32
    bf16 = mybir.dt.bfloat16
    nc = bacc.Bacc("TRN2", target_bir_lowering=False, debug=False, num_devices=M)
    ner = nc.dram_tensor("ner", [R, D], bf16, kind="ExternalInput").ap()
    face = nc.dram_tensor("face", [R, D], bf16, kind="ExternalInput").ap()
    sel = nc.dram_tensor("sel", [128, BL], bf16, kind="ExternalInput").ap()
    fm = nc.dram_tensor("fm", [BL, D], f32, kind="ExternalOutput").ap()
    nm = nc.dram_tensor("nm", [BL, D], f32, kind="ExternalOutput").ap()
    with tile.TileContext(nc) as tc:
        with ExitStack() as ctx:
            _emit_a(ctx, tc, fm, nm, ner, face, sel)
    nc.compile()
    return nc


def _build_b():
    import concourse.tile as tile
    from concourse import bacc, mybir

    f32 = mybir.dt.float32
    nc = bacc.Bacc("TRN2", target_bir_lowering=False, debug=False, num_devices=M)
    bf16 = mybir.dt.bfloat16
    fmt_full = nc.dram_tensor("fmt_full", [128, KD, B], bf16, kind="ExternalInput").ap()
    nmt = nc.dram_tensor("nmt", [128, KD * BL], bf16, kind="ExternalInput").ap()
    out = nc.dram_tensor("out", [BL, 128], f32, kind="ExternalOutput").ap()
    with tile.TileContext(nc) as tc:
        with ExitStack() as ctx:
            _emit_b(ctx, tc, out, fmt_full, nmt)
    nc.compile()
    return nc


def get_nc_a():
    if "a" not in _CACHE:
        _CACHE["a"] = _build_a()
    return _CACHE["a"]


def get_nc_b():
    if "b" not in _CACHE:
        _CACHE["b"] = _build_b()
    return _CACHE["b"]


def build_in_maps_a(face_j, ner_j):
    bf16 = ml_dtypes.bfloat16
    face_j = np.asarray(face_j, dtype=np.float32).astype(bf16)
    ner_j = np.asarray(ner_j, dtype=np.float32).astype(bf16)
    sel = np.zeros((128, BL), bf16)
    sel[np.arange(128), np.arange(128) // 4] = np.float32(1.0 / N1)
    return [
        {
            "ner": np.ascontiguousarray(ner_j[c * BL:(c + 1) * BL].reshape(R, D)),
            "face": np.ascontiguousarray(face_j[c * BL:(c + 1) * BL].reshape(R, D)),
            "sel": sel,
        }
        for c in range(M)
    ]


def build_in_maps_b(results_a):
    # the gather/transpose of the exchanged 32x512 means happens on host:
    # fmt_full[d', k, 32c + i] = fm_c[i, 128k + d']; nmt[d', 32k + i] = nm_c[i, 128k + d']
    bf16 = ml_dtypes.bfloat16
    F = np.stack([r["fm"] for r in results_a])          # [c, i, d]
    fmt_full = np.ascontiguousarray(
        F.reshape(M, BL, KD, 128).transpose(3, 2, 0, 1).reshape(128, KD, B)
    ).astype(bf16)
    return [
        {
            "fmt_full": fmt_full,
            "nmt": np.ascontiguousarray(
                results_a[c]["nm"].reshape(BL, KD, 128).transpose(2, 1, 0).reshape(128, KD * BL)
            ).astype(bf16),
        }
        for c in range(M)
    ]


def host_diag(results_a):
    # diag logit for core c's rows: <nm_c[i], fm_c[i]>
    return np.concatenate(
        [(results_a[c]["fm"] * results_a[c]["nm"]).sum(axis=1) for c in range(M)]
    )


def combine(results_a, results_b):
    diag = host_diag(results_a)
    rsum = np.concatenate([r["out"][:, 0] for r in results_b])
    return np.asarray(-np.mean(diag - np.log(rsum)), dtype=np.float32)


def _ensure_ntff_hook():
    """The agent image's antenv lacks axon_hooks; synthesize it and register the
    ctypes NTFF hook from trn_agent_boot so trace=True profiling works."""
    import sys
    import types

    try:
        from antenv.axon_hooks import get_axon_ntff_profile_hook  # noqa: F401

        return
    except ImportError:
        pass
    import antenv
    from trn_agent_boot.trn_boot import _ntff_profile_via_ctypes

    mod = types.ModuleType("antenv.axon_hooks")
    state = {"hook": None}
    mod.set_axon_ntff_profile_hook = lambda h: state.__setitem__("hook", h)
    mod.get_axon_ntff_profile_hook = lambda: state["hook"]
    sys.modules["antenv.axon_hooks"] = mod
    antenv.axon_hooks = mod
    mod.set_axon_ntff_profile_hook(_ntff_profile_via_ctypes("/opt/axon/libaxon_pjrt.so"))


def run_stage(nc, in_maps, trace=False, **kw):
    from concourse import bass_utils

    if trace:
        _ensure_ntff_hook()
    return bass_utils.run_bass_kernel_spmd(
        nc, in_maps, core_ids=list(range(M)), trace=trace, **kw
    )


def kernel(face_j, ner_j):
    res_a = run_stage(get_nc_a(), build_in_maps_a(face_j, ner_j))
    res_b = run_stage(get_nc_b(), build_in_maps_b(res_a.results))
    return combine(res_a.results, res_b.results)



# revision 3
# speedup vs baseline: 1.3439x; 1.3439x over previous
"""Trainium2 Bass kernel for nn_BatchSoftmaxNomax (batch contrastive softmax loss).

Math: scores[b,c,n,f] = <ner[b,n,:], face[c,f,:]>, logits = scores.mean((n,f)),
loss = -mean_b log_softmax(logits)[b,b].
Since the span-means are linear, logits[b,c] = <mean_n ner[b], mean_f face[c]>,
so the O(B^2*N^2*D) einsum collapses to two mean-reductions + a [B,D]x[D,B] matmul.

Sharding: ONE launch, d-sharded. Core c owns a 64-dim slice of D and computes the
partial logit matrix P_c[b,c'] = sum_{d in slice} nm[d,b]*fm[d,c'] for the FULL
batch; the host sums the 8 partials (the unshard step) and takes softmax/diag/mean.
A single launch avoids paying the Tile drain+sem-teardown tail (and program load)
twice, which dominated the two-launch layout.

Per core:
- Host packs X [128, 2, 4096] bf16: partitions 0:64 = ner[:, :, dsl], 64:128 =
  face[:, :, dsl], transposed to [d, n-half, b, n-sub] so each of the two
  "waves" is contiguous (2 KiB partition lines per chunk DMA).
- Wave 0 chunks stream over the HWDGE rings (sync/scalar); wave 1 chunks are
  SWDGE (gpsimd) DMAs with accum_op=add -- the SDMA CCE ALU does the first
  n-reduction step in the DMA path for free (HBM bytes unchanged).
- Remaining n=16 -> 1 span-sum: 4-level tensor_tensor add tree on DVE (bf16 2x),
  per chunk, into M[:, bq]. M [128, 256]: rows 0:64 = nmT sums, 64:128 = fmT sums.
- fmT lives on partitions 64:128 but the logits matmul contracts over partitions,
  so both operands must be co-located: relocate fmT down via a PE identity matmul
  (stationary = I/1024 at partitions 64:128 -- also folds in the double 1/32
  span-mean scaling) -> psum -> copy to fmr (bf16).
- Logits: 2 matmuls: lhsT = M[0:64, b-half] (stationary [64,128]), rhs = fmr
  [64, 256] -> psum [128 b, 256 c] fp32, copied to O [128, 512] bf16, one DMA out.

Host: P_c = O reshaped to [256, 256]; logits = sum_c P_c; loss from log_softmax.
"""

import ml_dtypes
import numpy as np
from contextlib import ExitStack

B = 256      # global batch
N = 32       # spans (N1 == N2)
D = 512      # embed dim
M = 8        # cores
DS = D // M  # d-dims per core (64)
NW = 2       # DMA-accumulated waves (n-halves)
NH = N // NW  # spans left after the DMA accumulate (16)
NCH = 4      # chunks per wave
FCH = B * NH // NCH  # free-dim cols per chunk (1024)
BCH = B // NCH       # b's per chunk (64)

_CACHE = {}


def _emit(ctx, tc, out, xin, ident):
    from concourse import mybir

    nc = tc.nc
    f32 = mybir.dt.float32
    bf16 = mybir.dt.bfloat16
    add = mybir.AluOpType.add

    consts = ctx.enter_context(tc.tile_pool(name="consts", bufs=1))
    data = ctx.enter_context(tc.tile_pool(name="data", bufs=1))
    work = ctx.enter_context(tc.tile_pool(name="work", bufs=1))
    scratch = ctx.enter_context(tc.tile_pool(name="scratch", bufs=2))
    psum = ctx.enter_context(tc.tile_pool(name="psum", bufs=1, space="PSUM"))

    # Identity/1024 [64, 64] must sit at partitions 64:128 to co-locate with fmT
    # (matmul operands share their partition range).
    idt = consts.tile([128, DS], bf16)
    nc.scalar.dma_start(idt[64:128, :], ident)

    queues = [nc.sync, nc.scalar]
    mt = work.tile([128, B], bf16)
    for q in range(NCH):
        t = data.tile([128, FCH], bf16, tag=f"x{q}")
        fsl = slice(q * FCH, (q + 1) * FCH)
        queues[q % 2].dma_start(t[:], xin[:, 0:1, fsl])
        # CCE add in the SDMA path: t += wave-1 chunk (SWDGE only).
        nc.gpsimd.dma_start(t[:], xin[:, 1:2, fsl], accum_op=add)

        # n=16 -> 1 add tree on DVE; bf16 keeps the 2x perf mode.
        v = t[:].rearrange("p (b n) -> p b n", n=NH)
        l1 = scratch.tile([128, BCH * 8], bf16, tag="l1")
        v1 = l1[:].rearrange("p (b n) -> p b n", n=8)
        nc.vector.tensor_tensor(v1, v[:, :, 0:8], v[:, :, 8:16], op=add)
        l2 = scratch.tile([128, BCH * 4], bf16, tag="l2")
        v2 = l2[:].rearrange("p (b n) -> p b n", n=4)
        nc.vector.tensor_tensor(v2, v1[:, :, 0:4], v1[:, :, 4:8], op=add)
        l3 = scratch.tile([128, BCH * 2], bf16, tag="l3")
        v3 = l3[:].rearrange("p (b n) -> p b n", n=2)
        nc.vector.tensor_tensor(v3, v2[:, :, 0:2], v2[:, :, 2:4], op=add)
        nc.vector.tensor_tensor(
            mt[:, q * BCH:(q + 1) * BCH], v3[:, :, 0:1], v3[:, :, 1:2], op=add
        )

    # Relocate fmT (partitions 64:128) down to 0:64 through the PE; the
    # stationary I/1024 also applies the (1/32)^2 span-mean normalization.
    ps_f = psum.tile([DS, B], f32)
    nc.tensor.matmul(ps_f[:], idt[64:128, :], mt[64:128, :], start=True, stop=True)
    fmr = work.tile([DS, B], bf16)
    nc.vector.tensor_copy(fmr[:], ps_f[:])

    # Partial logits, b in two 128-row halves.
    ob = work.tile([128, 2 * B], bf16)
    for h in range(2):
        lg = psum.tile([128, B], f32, tag=f"lg{h}")
        nc.tensor.matmul(lg[:], mt[0:DS, h * 128:(h + 1) * 128], fmr[:],
                         start=True, stop=True)
        nc.vector.tensor_copy(ob[:, h * B:(h + 1) * B], lg[:])
    nc.sync.dma_start(out, ob[:])


def _build():
    import concourse.tile as tile
    from concourse import bacc, mybir

    bf16 = mybir.dt.bfloat16
    nc = bacc.Bacc("TRN2", target_bir_lowering=False, debug=False, num_devices=M)
    xin = nc.dram_tensor("xin", [128, NW, B * NH], bf16, kind="ExternalInput").ap()
    ident = nc.dram_tensor("ident", [DS, DS], bf16, kind="ExternalInput").ap()
    out = nc.dram_tensor("out", [128, 2 * B], bf16, kind="ExternalOutput").ap()
    with tile.TileContext(nc) as tc:
        with ExitStack() as ctx:
            _emit(ctx, tc, out, xin, ident)
    nc.compile()
    return nc


def get_nc():
    if "nc" not in _CACHE:
        _CACHE["nc"] = _build()
    return _CACHE["nc"]


def build_in_maps(face_j, ner_j):
    bf16 = ml_dtypes.bfloat16
    face_j = np.asarray(face_j, dtype=np.float32)
    ner_j = np.asarray(ner_j, dtype=np.float32)
    ident = (np.eye(DS, dtype=np.float32) / (N * N)).astype(bf16)
    maps = []
    for c in range(M):
        dsl = slice(c * DS, (c + 1) * DS)
        # [d, b, n] -> [d, n-wave, b, n-sub] -> [d, NW, B*NH] per tensor.
        a = ner_j[:, :, dsl].transpose(2, 0, 1).reshape(DS, B, NW, NH)
        a = a.transpose(0, 2, 1, 3).reshape(DS, NW, B * NH)
        b = face_j[:, :, dsl].transpose(2, 0, 1).reshape(DS, B, NW, NH)
        b = b.transpose(0, 2, 1, 3).reshape(DS, NW, B * NH)
        xin = np.ascontiguousarray(np.concatenate([a, b], axis=0)).astype(bf16)
        maps.append({"xin": xin, "ident": ident})
    return maps


def combine(results):
    # Unshard: sum the per-core partial logit matrices, then the softmax loss.
    logits = np.zeros((B, B), dtype=np.float64)
    for r in results:
        o = np.asarray(r["out"], dtype=np.float64)  # [128, 512]
        logits[0:128] += o[:, 0:B]
        logits[128:256] += o[:, B:2 * B]
    lse = np.log(np.exp(logits).sum(axis=1))
    diag = np.diagonal(logits)
    return np.asarray(-(diag - lse).mean(), dtype=np.float32)


def _ensure_ntff_hook():
    """The agent image's antenv lacks axon_hooks; synthesize it and register the
    ctypes NTFF hook from trn_agent_boot so trace=True profiling works."""
    import sys
    import types

    try:
        from antenv.axon_hooks import get_axon_ntff_profile_hook  # noqa: F401

        return
    except ImportError:
        pass
    import antenv
    from trn_agent_boot.trn_boot import _ntff_profile_via_ctypes

    mod = types.ModuleType("antenv.axon_hooks")
    state = {"hook": None}
    mod.set_axon_ntff_profile_hook = lambda h: state.__setitem__("hook", h)
    mod.get_axon_ntff_profile_hook = lambda: state["hook"]
    sys.modules["antenv.axon_hooks"] = mod
    antenv.axon_hooks = mod
    mod.set_axon_ntff_profile_hook(_ntff_profile_via_ctypes("/opt/axon/libaxon_pjrt.so"))


def run_stage(nc, in_maps, trace=False, **kw):
    from concourse import bass_utils

    if trace:
        _ensure_ntff_hook()
    return bass_utils.run_bass_kernel_spmd(
        nc, in_maps, core_ids=list(range(M)), trace=trace, **kw
    )


def kernel(face_j, ner_j):
    res = run_stage(get_nc(), build_in_maps(face_j, ner_j))
    return combine(res.results)


# revision 7
# speedup vs baseline: 1.3854x; 1.0308x over previous
"""Trainium2 Bass kernel for nn_BatchSoftmaxNomax (batch contrastive softmax loss).

Math: scores[b,c,n,f] = <ner[b,n,:], face[c,f,:]>, logits = scores.mean((n,f)),
loss = -mean_b log_softmax(logits)[b,b].
Since the span-means are linear, logits[b,c] = <mean_n ner[b], mean_f face[c]>,
so the O(B^2*N^2*D) einsum collapses to two mean-reductions + a [B,D]x[D,B] matmul.

Sharding: ONE launch, d-sharded. Core c owns a 64-dim slice of D and computes the
partial logit matrix P_c[b,c'] = sum_{d in slice} nm[d,b]*fm[d,c'] for the FULL
batch; the host sums the 8 partials (the unshard step) and takes softmax/diag/mean.
A single launch avoids paying the Tile drain+sem-teardown tail (and program load)
twice, which dominated the two-launch layout.

Per core:
- Host packs X [128, 2, 4096] bf16: partitions 0:64 = ner[:, :, dsl], 64:128 =
  face[:, :, dsl], transposed to [d, n-half, b, n-sub] so each of the two
  "waves" is contiguous (2 KiB partition lines per chunk DMA).
- Wave 0 chunks stream over the HWDGE rings (sync/scalar); wave 1 chunks are
  SWDGE (gpsimd) DMAs with accum_op=add -- the SDMA CCE ALU does the first
  n-reduction step in the DMA path for free (HBM bytes unchanged).
- Remaining n=16 -> 1 span-sum: 4-level tensor_tensor add tree on DVE (bf16 2x),
  per chunk, into M[:, bq]. M [128, 256]: rows 0:64 = nmT sums, 64:128 = fmT sums.
- fmT lives on partitions 64:128 but the logits matmul contracts over partitions,
  so both operands must be co-located: relocate fmT down via a PE identity matmul
  (stationary = I/1024 at partitions 64:128 -- also folds in the double 1/32
  span-mean scaling) -> psum -> copy to fmr (bf16).
- Logits: 2 matmuls: lhsT = M[0:64, b-half] (stationary [64,128]), rhs = fmr
  [64, 256] -> psum [128 b, 256 c] fp32, copied to O [128, 512] bf16, one DMA out.

Host: P_c = O reshaped to [256, 256]; logits = sum_c P_c; loss from log_softmax.
"""

import ml_dtypes
import numpy as np
from contextlib import ExitStack

B = 256      # global batch
N = 32       # spans (N1 == N2)
D = 512      # embed dim
M = 8        # cores
DS = D // M  # d-dims per core (64)
NDMA = 2     # input DMA chunks (big: 8 KiB partition lines for descriptor eff.)
NCH = 4      # reduction sub-chunks (b-quarters)
FCH = B * N // NCH   # free-dim cols per sub-chunk (2048)
BCH = B // NCH       # b's per sub-chunk (64)

_CACHE = {}


def _emit(ctx, tc, out, xin, ident):
    from concourse import mybir

    nc = tc.nc
    f32 = mybir.dt.float32
    bf16 = mybir.dt.bfloat16
    add = mybir.AluOpType.add

    consts = ctx.enter_context(tc.tile_pool(name="consts", bufs=1))
    data = ctx.enter_context(tc.tile_pool(name="data", bufs=1))
    work = ctx.enter_context(tc.tile_pool(name="work", bufs=1))
    scratch = ctx.enter_context(tc.tile_pool(name="scratch", bufs=2))
    psum = ctx.enter_context(tc.tile_pool(name="psum", bufs=1, space="PSUM"))

    # Identity/1024 [64, 64] must sit at partitions 64:128 to co-locate with fmT
    # (matmul operands share their partition range).
    idt = consts.tile([128, DS], bf16)
    nc.scalar.dma_start(idt[64:128, :], ident)

    queues = [nc.sync, nc.scalar]
    # Two big input DMAs (one per HWDGE ring): [128, 4096] bf16 = 8 KiB
    # contiguous per partition line, the knee of descriptor efficiency.
    xts = []
    dchunk = B * N // NDMA
    for i in range(NDMA):
        t = data.tile([128, dchunk], bf16, tag=f"x{i}")
        queues[i % 2].dma_start(t[:], xin[:, i * dchunk:(i + 1) * dchunk])
        xts.append(t)

    mt = work.tile([128, B], bf16)
    for q in range(NCH):
        # n=32 -> 1 add tree on DVE (bf16 keeps the 2x perf mode), one
        # b-quarter at a time so trees overlap the second input DMA.
        t = xts[q * NDMA // NCH]
        base = (q * FCH) % dchunk
        v = t[:, base:base + FCH].rearrange("p (b n) -> p b n", n=N)
        l1 = scratch.tile([128, BCH * 16], bf16, tag="l1")
        v1 = l1[:].rearrange("p (b n) -> p b n", n=16)
        nc.vector.tensor_tensor(v1, v[:, :, 0:16], v[:, :, 16:32], op=add)
        l2 = scratch.tile([128, BCH * 8], bf16, tag="l2")
        v2 = l2[:].rearrange("p (b n) -> p b n", n=8)
        nc.vector.tensor_tensor(v2, v1[:, :, 0:8], v1[:, :, 8:16], op=add)
        l3 = scratch.tile([128, BCH * 4], bf16, tag="l3")
        v3 = l3[:].rearrange("p (b n) -> p b n", n=4)
        nc.vector.tensor_tensor(v3, v2[:, :, 0:4], v2[:, :, 4:8], op=add)
        l4 = scratch.tile([128, BCH * 2], bf16, tag="l4")
        v4 = l4[:].rearrange("p (b n) -> p b n", n=2)
        nc.vector.tensor_tensor(v4, v3[:, :, 0:2], v3[:, :, 2:4], op=add)
        nc.vector.tensor_tensor(
            mt[:, q * BCH:(q + 1) * BCH], v4[:, :, 0:1], v4[:, :, 1:2], op=add
        )

    # Relocate fmT (partitions 64:128) down to 0:64 through the PE; the
    # stationary I/1024 also applies the (1/32)^2 span-mean normalization.
    ps_f = psum.tile([DS, B], f32)
    nc.tensor.matmul(ps_f[:], idt[64:128, :], mt[64:128, :], start=True, stop=True)
    fmr = work.tile([DS, B], bf16)
    nc.vector.tensor_copy(fmr[:], ps_f[:])

    # Partial logits, b in two 128-row halves.
    ob = work.tile([128, 2 * B], bf16)
    for h in range(2):
        lg = psum.tile([128, B], f32, tag=f"lg{h}")
        nc.tensor.matmul(lg[:], mt[0:DS, h * 128:(h + 1) * 128], fmr[:],
                         start=True, stop=True)
        nc.vector.tensor_copy(ob[:, h * B:(h + 1) * B], lg[:])
    nc.sync.dma_start(out, ob[:])


def _build():
    import concourse.tile as tile
    from concourse import bacc, mybir

    bf16 = mybir.dt.bfloat16
    nc = bacc.Bacc("TRN2", target_bir_lowering=False, debug=False, num_devices=M)
    xin = nc.dram_tensor("xin", [128, B * N], bf16, kind="ExternalInput").ap()
    ident = nc.dram_tensor("ident", [DS, DS], bf16, kind="ExternalInput").ap()
    out = nc.dram_tensor("out", [128, 2 * B], bf16, kind="ExternalOutput").ap()
    with tile.TileContext(nc) as tc:
        with ExitStack() as ctx:
            _emit(ctx, tc, out, xin, ident)
    nc.compile()
    return nc


def get_nc():
    if "nc" not in _CACHE:
        _CACHE["nc"] = _build()
    return _CACHE["nc"]


def build_in_maps(face_j, ner_j):
    bf16 = ml_dtypes.bfloat16
    face_j = np.asarray(face_j, dtype=np.float32)
    ner_j = np.asarray(ner_j, dtype=np.float32)
    ident = (np.eye(DS, dtype=np.float32) / (N * N)).astype(bf16)
    maps = []
    for c in range(M):
        dsl = slice(c * DS, (c + 1) * DS)
        # [d, b, n] flattened to [64, 8192] per tensor, stacked on partitions.
        a = ner_j[:, :, dsl].transpose(2, 0, 1).reshape(DS, B * N)
        b = face_j[:, :, dsl].transpose(2, 0, 1).reshape(DS, B * N)
        xin = np.ascontiguousarray(np.concatenate([a, b], axis=0)).astype(bf16)
        maps.append({"xin": xin, "ident": ident})
    return maps


def combine(results):
    # Unshard: sum the per-core partial logit matrices, then the softmax loss.
    logits = np.zeros((B, B), dtype=np.float64)
    for r in results:
        o = np.asarray(r["out"], dtype=np.float64)  # [128, 512]
        logits[0:128] += o[:, 0:B]
        logits[128:256] += o[:, B:2 * B]
    lse = np.log(np.exp(logits).sum(axis=1))
    diag = np.diagonal(logits)
    return np.asarray(-(diag - lse).mean(), dtype=np.float32)


def _ensure_ntff_hook():
    """The agent image's antenv lacks axon_hooks; synthesize it and register the
    ctypes NTFF hook from trn_agent_boot so trace=True profiling works."""
    import sys
    import types

    try:
        from antenv.axon_hooks import get_axon_ntff_profile_hook  # noqa: F401

        return
    except ImportError:
        pass
    import antenv
    from trn_agent_boot.trn_boot import _ntff_profile_via_ctypes

    mod = types.ModuleType("antenv.axon_hooks")
    state = {"hook": None}
    mod.set_axon_ntff_profile_hook = lambda h: state.__setitem__("hook", h)
    mod.get_axon_ntff_profile_hook = lambda: state["hook"]
    sys.modules["antenv.axon_hooks"] = mod
    antenv.axon_hooks = mod
    mod.set_axon_ntff_profile_hook(_ntff_profile_via_ctypes("/opt/axon/libaxon_pjrt.so"))


def run_stage(nc, in_maps, trace=False, **kw):
    from concourse import bass_utils

    if trace:
        _ensure_ntff_hook()
    return bass_utils.run_bass_kernel_spmd(
        nc, in_maps, core_ids=list(range(M)), trace=trace, **kw
    )


def kernel(face_j, ner_j):
    res = run_stage(get_nc(), build_in_maps(face_j, ner_j))
    return combine(res.results)
